# revision 2
# baseline (speedup 1.0000x reference)
"""Multi-head causal attention (B=2, S=2048, D=1024, H=16) on 8 TRN2 cores.

Sharding: tensor-parallel over heads. Core c owns heads {2c, 2c+1} and rows
[128c, 128c+128) of Wo. Each core computes its heads' attention and the
partial output projection; the host sums the 8 partials (the "all-reduce")
and adds the bias.

v2: Q/K projections run in fp8(e4m3) DoubleRow perf mode (K=256 per matmul,
M=64, 0.5 cycles/column -> 2x PE throughput); x for the V projection streams
in group-major layout (xTg) through a small SBUF ring so V-filler matmuls
never wait on bulk DMA and SBUF fits the extra fp8 copy of x.

Device layout:
  xT8     [128, 4, 2, BS] fp8  x^T in d-chunk-pair layout for DoubleRow
  xTg     ring of [128, 8, 128] bf16 group tiles (V projection input)
  wq8/wk8 [128, 4, 2, 2, 64] fp8  pair-layout per local head
  wv      [128, 8, 128] bf16; wo [128, 1024] bf16
  qT/kT   [128, 4096] bf16; V_sb [128, 32, 2, 128] bf16; OT [128, 4096]

Per core:
  1. Q^T/K^T = fp8 DoubleRow (w.T @ xT), batch 0 inline, batch 1 as filler;
     V in [t, k] layout with a leading ones column (softmax denominators
     emitted by the attention matmul into PSUM partition 0).
  2. Causal attention in scores^T orientation, quarter-major deferred AV
     (see v1 docstring): exp stream on ACT, dense AV bursts, fast
     reciprocal + K=1 broadcast matmul normalize.
  3. Partial projection out_pT = wo_rows.T @ OT interleaved as PE filler.
"""

import numpy as np
import ml_dtypes

B, S, D, H = 2, 2048, 1024, 16
HD = 64          # head dim
NCORES = 8
HL = H // NCORES  # local heads per core = 2
BS = B * S        # 4096
SCALE = float(D) ** -0.5

BF16 = ml_dtypes.bfloat16
E4M3 = ml_dtypes.float8_e4m3fn

_CACHE = {}


def _build_kernel():
    import concourse.mybir as mybir
    import concourse.tile as tile
    from concourse import bacc

    bf16 = mybir.dt.bfloat16
    fp8 = mybir.dt.float8e4
    f32 = mybir.dt.float32
    Exp = mybir.ActivationFunctionType.Exp
    DR = mybir.MatmulPerfMode.DoubleRow

    nc = bacc.Bacc("TRN2", debug=False, enable_asserts=False)
    xT8_d = nc.dram_tensor("xT8", [128, 4, 2, BS], fp8, kind="ExternalInput").ap()
    xTg_d = nc.dram_tensor("xTg", [128, BS // 128, 8, 128], bf16, kind="ExternalInput").ap()
    wq8_d = nc.dram_tensor("wq8", [128, 4, 2, 128], fp8, kind="ExternalInput").ap()
    wk8_d = nc.dram_tensor("wk8", [128, 4, 2, 128], fp8, kind="ExternalInput").ap()
    wv_d = nc.dram_tensor("wv", [D, 128], bf16, kind="ExternalInput").ap()
    wo_d = nc.dram_tensor("wo", [128, D], bf16, kind="ExternalInput").ap()
    # consts: cols 0:128 = upper-tri mask (1 where col >= row), cols 128:192 =
    # 64x64 identity replicated in both partition halves.
    consts_d = nc.dram_tensor("consts", [128, 192], bf16, kind="ExternalInput").ap()
    out_d = nc.dram_tensor("out_pT", [D, BS], bf16, kind="ExternalOutput").ap()

    DC = D // 128   # 8 d-chunks
    OP = DC // 2    # 4 d-chunk pairs for DoubleRow
    NT = S // 128   # 16 key blocks per sequence
    NG = BS // 128  # 32 V groups

    with tile.TileContext(nc) as tc:
        with tc.tile_pool(name="persist", bufs=1) as pp:
            xT8 = pp.tile([128, OP, 2, BS], fp8, tag="xT8")
            qT = pp.tile([128, BS], bf16, tag="qT")
            kT = pp.tile([128, BS], bf16, tag="kT")
            # V in [t, k] layout, padded to 128 columns: col 0 = 1.0 (the
            # ones column makes the attention matmul emit softmax
            # denominators in PSUM partition 0), cols 1:64 = 0, cols
            # 64:128 = V block for s-block g and local head j.
            V_sb = pp.tile([128, NG, HL, 128], bf16, tag="V")
            OT = pp.tile([128, BS], bf16, tag="OT")
            wq8 = pp.tile([128, OP, 2, 128], fp8, tag="wq8")
            wk8 = pp.tile([128, OP, 2, 128], fp8, tag="wk8")
            wv = pp.tile([128, DC, 128], bf16, tag="wv")
            wo = pp.tile([128, D], bf16, tag="wo")
            consts = pp.tile([128, 192], bf16, tag="consts")
            trimask = consts[:, 0:128]
            ident = consts[:, 128:192]  # noqa: F841 (kept for layout parity)
            ones64 = pp.tile([1, 64], f32, tag="ones64")

            # DMA in. The phase-1 critical set (wk8, first xT8 columns, wq8)
            # issues from the ACT queue -- descriptor writes on the sync
            # engine are ~0.6us each and serialize startup otherwise. ACT's
            # exp warmup is emitted after these so it doesn't block them
            # (it waits on consts, which arrives via sync).
            nc.scalar.dma_start(wk8[:], wk8_d[:])
            for op in range(OP):
                nc.scalar.dma_start(xT8[:, op, :, 0:512], xT8_d[:, op, :, 0:512])
            nc.scalar.dma_start(wq8[:], wq8_d[:])
            for op in range(OP):
                nc.scalar.dma_start(xT8[:, op, :, 512:2048], xT8_d[:, op, :, 512:2048])
            # sync queue: consts + V-projection inputs early (first V filler
            # runs right after attention starts), bulk batch-1 x later.
            nc.sync.dma_start(consts[:], consts_d[:])
            nc.sync.dma_start(wv[:], wv_d.rearrange("(o p) c -> p o c", p=128))
            nc.vector.memset(ones64[:], 1.0)
            # Preheat the ACT exp table so the first real exp doesn't pay
            # the table-load latency mid-pipeline.
            warmup = pp.tile([1, 8], bf16, tag="warmup")
            nc.scalar.activation(warmup[:], consts[0:1, 0:8], Exp, scale=SCALE)

            # ---- Phase 1: Q^T / K^T fp8 DoubleRow projections, batch 0 ----
            # Batch 1's projection chunks are deferred into the attention
            # loop as filler (they're only needed from the third head on).
            # unit order: kT s0 first (first scores block needs it), then
            # qT s0..s3 (rhs of the first scores), then the rest of kT; each
            # unit copies to SBUF immediately so attention starts gap-free.
            with tc.tile_pool(name="ph1psum", bufs=8, space="PSUM") as ph1:
                units = [(wk8, kT, 0), (wq8, qT, 0), (wq8, qT, 1),
                         (wq8, qT, 2), (wq8, qT, 3), (wk8, kT, 1),
                         (wk8, kT, 2), (wk8, kT, 3)]
                for ui, (w_sb, dst, s) in enumerate(units):
                    ps1 = ph1.tile([128, 512], f32, tag="ph1", name="ph1")
                    for op in range(OP):
                        nc.tensor.matmul(
                            ps1[:],
                            lhsT=w_sb[:, op, :, :],
                            rhs=xT8[:, op, :, 512 * s : 512 * (s + 1)],
                            start=(op == 0),
                            stop=(op == OP - 1),
                            perf_mode=DR,
                        )
                    if ui < 3:
                        nc.scalar.copy(dst[:, 512 * s : 512 * (s + 1)], ps1[:])
                    else:
                        nc.vector.tensor_copy(dst[:, 512 * s : 512 * (s + 1)], ps1[:])

            # V_sb constants; emitted after phase-1 so these DVE ops
            # don't delay the urgent qT/kT copies.
            nc.vector.memset(V_sb[:, :, :, 0:HD], 0.0)
            nc.vector.memset(V_sb[:, :, :, 0:1], 1.0)

            # ---- Phases 2+3: V projection (streamed xTg), attention, and
            # output projection, all interleaved on the PE. ----
            with (
                tc.tile_pool(name="po", bufs=2, space="PSUM") as po_pool,
                tc.tile_pool(name="ps", bufs=2, space="PSUM") as ps_pool,
                tc.tile_pool(name="aux", bufs=2, space="PSUM") as aux_pool,
                tc.tile_pool(name="expp", bufs=26) as exp_pool,
                tc.tile_pool(name="xtg", bufs=8) as xtg_pool,
                tc.tile_pool(name="recip", bufs=3) as rc_pool,
                tc.tile_pool(name="onum", bufs=3) as on_pool,
                tc.tile_pool(name="ph4out", bufs=2) as ph4o,
            ):
                xtg_tiles = {}

                def fetch_group(g):
                    if g >= NG:
                        return
                    t = xtg_pool.tile([128, DC, 128], bf16, tag="xtg", name="xtg")
                    nc.sync.dma_start(t[:], xTg_d[:, g, :, :])
                    xtg_tiles[g] = t

                def emit_v(vps, g, tag="pv"):
                    # V directly in [t, k] layout: lhsT = xTg s-block
                    # (stationary), rhs = wv: psum[s, c] = V block for both
                    # local heads side by side.
                    fetch_group(g + 6)
                    src = xtg_tiles.pop(g)
                    pv = vps.tile([128, 128], f32, tag=tag, name="pv")
                    for o in range(DC):
                        nc.tensor.matmul(
                            pv[:],
                            lhsT=src[:, o, :],
                            rhs=wv[:, o, :],
                            start=(o == 0),
                            stop=(o == DC - 1),
                        )
                    nc.vector.tensor_copy(
                        V_sb[:, g, :, HD : 2 * HD],
                        pv[:].rearrange("p (j k) -> p j k", j=HL),
                    )

                # prefetch the first V groups before attention begins,
                # then queue the later-needed bulk inputs behind them
                for g in range(6):
                    fetch_group(g)
                nc.sync.dma_start(wo[:], wo_d[:])
                for op in range(OP):
                    nc.sync.dma_start(
                        xT8[:, op, :, 2048:4096], xT8_d[:, op, :, 2048:4096])

                # ---- causal attention per (b, local head) ----
                # Quarter-major deferred AV (see v1): scores/exp stream in kj
                # order, every exp tile kept in SBUF, each 512-wide o^T
                # quarter accumulated as one dense burst then normalized.
                def emit_block(b, j, kj, p0, p1):
                    qTh = qT[64 * j : 64 * (j + 1), S * b : S * (b + 1)]
                    kTh = kT[64 * j : 64 * (j + 1), S * b : S * (b + 1)]
                    s_lo = 128 * kj
                    w = p1 - p0
                    ps = ps_pool.tile([128, 1024], f32, tag="ps", name="ps")
                    for c0 in range(0, w, 512):
                        c1 = min(c0 + 512, w)
                        nc.tensor.matmul(
                            ps[:, c0:c1],
                            lhsT=kTh[:, s_lo : s_lo + 128],
                            rhs=qTh[:, p0 + c0 : p0 + c1],
                            start=True,
                            stop=True,
                        )
                    et = exp_pool.tile([128, 1024], bf16, tag="expT", name="et")
                    nc.scalar.activation(et[:, 0:w], ps[:, 0:w], Exp, scale=SCALE)
                    if p0 == s_lo:
                        # diagonal 128x128: keep only s' >= t
                        nc.vector.tensor_mul(et[:, 0:128], et[:, 0:128], trimask[:])
                    return et

                def emit_quarter(b, j, q, ets):
                    kj_last = 4 * q + 3
                    pq = po_pool.tile([128, 512], f32, tag="po", name="pq")
                    for k2 in range(kj_last + 1):
                        a0 = max(512 * q, 128 * k2)
                        a1 = 512 * (q + 1)
                        for p0, p1, et in ets[k2]:
                            if p0 <= a0 < p1:
                                break
                        else:
                            raise AssertionError("no piece")
                        nc.tensor.matmul(
                            pq[:, a0 - 512 * q : a1 - 512 * q],
                            lhsT=V_sb[:, NT * b + k2, j, :],
                            rhs=et[:, a0 - p0 : a1 - p0],
                            start=(k2 == 0),
                            stop=(k2 == kj_last),
                        )
                    return pq

                def emit_normalize(b, j, q, pq):
                    # o^T[k, s] / denom[s] for quarter q. Copy the numerator
                    # to SBUF (frees the po slot), fast-reciprocal the
                    # denominator row (PSUM partition 0), broadcast it
                    # across 64 partitions via a K=1 matmul, then a single
                    # one-PSUM-operand multiply into OT.
                    onum = on_pool.tile([64, 512], f32, tag="onum", name="onum")
                    nc.scalar.copy(onum[:], pq[HD : 2 * HD, :])
                    rc = rc_pool.tile([1, 512], f32, tag="rc", name="rc")
                    nc.vector.reciprocal_approx_fast(rc[:], pq[0:1, :])
                    pb = aux_pool.tile([64, 512], f32, tag="aux", name="pb")
                    nc.tensor.matmul(pb[:], lhsT=ones64[:], rhs=rc[:], start=True, stop=True)
                    nc.vector.tensor_mul(
                        OT[64 * j : 64 * (j + 1),
                           S * b + 512 * q : S * b + 512 * (q + 1)],
                        onum[:],
                        pb[:],
                    )

                out_r = out_d.rearrange("(o p) s -> p o s", p=128)

                def emit_ph4_nb(b, nb, rush=False):
                    # partial projection for 512 columns of batch b:
                    # out_pT[:, cols] = wo.T @ OT[:, cols]. rush=True (the
                    # kernel's last chunk): two half-stages with the casts
                    # alternating ACT/DVE so the tail isn't serialized on
                    # one engine.
                    for half in range(2 if rush else 1):
                        dcs = range(half * 4, half * 4 + 4) if rush else range(DC)
                        stage = ph4o.tile([128, DC, 512], bf16, tag="o4", name="o4")
                        for i, dc in enumerate(dcs):
                            pp4 = aux_pool.tile([128, 512], f32, tag="aux", name="pp4")
                            nc.tensor.matmul(
                                pp4[:],
                                lhsT=wo[:, 128 * dc : 128 * (dc + 1)],
                                rhs=OT[:, S * b + 512 * nb : S * b + 512 * (nb + 1)],
                                start=True,
                                stop=True,
                            )
                            if rush and i % 2 == 0:
                                nc.scalar.copy(stage[:, dc, :], pp4[:])
                            else:
                                nc.vector.tensor_copy(stage[:, dc, :], pp4[:])
                        rows = (slice(half * 4, half * 4 + 4) if rush
                                else slice(0, DC))
                        nc.sync.dma_start(
                            out_r[:, rows, S * b + 512 * nb : S * b + 512 * (nb + 1)],
                            stage[:, rows, :],
                        )

                def emit_qk1(w_sb, dst, sc):
                    # one batch-1 projection chunk: 8 fp8 DoubleRow matmuls
                    pk = aux_pool.tile([128, 512], f32, tag="aux", name="pk")
                    for op in range(OP):
                        nc.tensor.matmul(
                            pk[:],
                            lhsT=w_sb[:, op, :, :],
                            rhs=xT8[:, op, :, 512 * sc : 512 * (sc + 1)],
                            start=(op == 0),
                            stop=(op == OP - 1),
                            perf_mode=DR,
                        )
                    nc.vector.tensor_copy(dst[:, 512 * sc : 512 * (sc + 1)], pk[:])

                filler_q = [("v", g) for g in range(0, BS // 256)]
                for sc in range(4, 8):
                    filler_q.append(("qk", wq8, qT, sc))
                    filler_q.append(("qk", wk8, kT, sc))
                filler_q += [("v", g) for g in range(BS // 256, BS // 128)]

                def emit_filler():
                    if not filler_q:
                        return
                    u = filler_q.pop(0)
                    if u[0] == "qk":
                        emit_qk1(u[1], u[2], u[3])
                    else:
                        emit_v(aux_pool, u[1], tag="aux")

                ph4_queue = []      # (b, nb) chunks awaiting emission
                ph4_state = None    # (b, nb, stage, next_dc)

                def emit_ph4_step():
                    # one dc-chunk of a pending output-projection unit
                    nonlocal ph4_state
                    if ph4_state is None:
                        if not ph4_queue:
                            return
                        b4, nb4 = ph4_queue.pop(0)
                        stage = ph4o.tile([128, DC, 512], bf16, tag="o4", name="o4")
                        ph4_state = (b4, nb4, stage, 0)
                    b4, nb4, stage, dc = ph4_state
                    pp4 = aux_pool.tile([128, 512], f32, tag="aux", name="pp4")
                    nc.tensor.matmul(
                        pp4[:],
                        lhsT=wo[:, 128 * dc : 128 * (dc + 1)],
                        rhs=OT[:, S * b4 + 512 * nb4 : S * b4 + 512 * (nb4 + 1)],
                        start=True,
                        stop=True,
                    )
                    nc.vector.tensor_copy(stage[:, dc, :], pp4[:])
                    if dc == DC - 1:
                        nc.sync.dma_start(
                            out_r[:, :, S * b4 + 512 * nb4 : S * b4 + 512 * (nb4 + 1)],
                            stage[:],
                        )
                        ph4_state = None
                    else:
                        ph4_state = (b4, nb4, stage, dc + 1)

                for bh in range(B * HL):
                    b, j = bh // HL, bh % HL
                    ets = {}
                    for kj in range(NT):
                        s_lo = 128 * kj
                        pieces = ([(s_lo, 1024), (1024, 2048)] if s_lo < 1024
                                  else [(s_lo, 2048)])
                        ets[kj] = [(p0, p1, emit_block(b, j, kj, p0, p1))
                                   for p0, p1 in pieces]
                        # quarter q's AV burst is deferred by one block so
                        # its final exp tile is ready when the burst reaches
                        # it (quarter 3 naturally gets this slack from the
                        # next head's first blocks)
                        if kj % 4 == 0 and kj > 0:
                            q = kj // 4 - 1
                            pq = emit_quarter(b, j, q, ets)
                            emit_normalize(b, j, q, pq)
                            if bh == 3:
                                ph4_queue.append((1, q))
                        if bh == 2 and kj % 4 == 0 and kj > 0:
                            # batch-0 output projection as bh2 filler (OT
                            # batch-0 columns completed during bh1),
                            # spread one dc-chunk per kj step below
                            ph4_queue.append((0, kj // 4 - 1))
                        if bh >= 2:
                            emit_ph4_step()
                            emit_ph4_step()
                        if bh <= 1:
                            # one filler unit per kj in bh0 (paced to the V
                            # deadlines), two per kj in bh1 to drain
                            emit_filler()
                            if bh == 1:
                                emit_filler()
                    q = 3
                    pq = emit_quarter(b, j, q, ets)
                    emit_normalize(b, j, q, pq)
                    if bh == 1:
                        # anything batch-1 still pending must land before bh2
                        while filler_q:
                            emit_filler()
                    if bh == 2:
                        ph4_queue.append((0, 3))
                    if bh == 3:
                        while ph4_queue or ph4_state is not None:
                            emit_ph4_step()
                        emit_ph4_nb(1, 3, rush=True)

    nc.compile()
    return nc


def get_nc():
    if "nc" not in _CACHE:
        _CACHE["nc"] = _build_kernel()
    return _CACHE["nc"]


def make_in_maps(x, Wq, Wk, Wv, Wo):
    """Host-side sharding: per-core input dict (numpy)."""
    x = np.asarray(x, np.float32)
    Wq = np.asarray(Wq, np.float32)
    Wk = np.asarray(Wk, np.float32)
    Wv = np.asarray(Wv, np.float32)
    Wo = np.asarray(Wo, np.float32)
    xT = np.ascontiguousarray(x.transpose(2, 0, 1).reshape(D, BS))
    # x quantized to bf16 first (the v1 baseline numerics), then to e4m3 for
    # the fp8 Q/K path so device and host casts agree.
    xTb = xT.astype(BF16)
    # xT8[p, op, i, s] = xT[128*(2*op+i)+p, s]
    xT8 = np.ascontiguousarray(
        xTb.astype(np.float32).reshape(4, 2, 128, BS).transpose(2, 0, 1, 3)
    ).astype(E4M3)
    # xTg[p, g, o, c] = xT[128*o+p, 128*g+c]
    xTg = np.ascontiguousarray(
        xTb.reshape(D // 128, 128, BS // 128, 128).transpose(1, 2, 0, 3)
    )

    in_maps = []
    for c in range(NCORES):
        h0 = HL * c

        def pack8(W):
            # [HL, D, HD] -> [128, 4, 2, 128] e4m3 pair layout, both heads
            # packed in the last dim (j*64+m)
            Wl = W[h0 : h0 + HL].astype(BF16).astype(np.float32)
            return np.ascontiguousarray(
                Wl.reshape(HL, 4, 2, 128, HD).transpose(3, 1, 2, 0, 4)
                .reshape(128, 4, 2, 128)
            ).astype(E4M3)

        wv_l = np.ascontiguousarray(
            Wv[h0 : h0 + HL].transpose(1, 0, 2).reshape(D, HL * HD)
        ).astype(BF16)
        in_maps.append(
            {
                "xT8": xT8,
                "xTg": xTg,
                "wq8": pack8(Wq),
                "wk8": pack8(Wk),
                "wv": wv_l,
                "wo": np.ascontiguousarray(Wo[128 * c : 128 * (c + 1), :]).astype(BF16),
                "consts": _make_consts(),
            }
        )
    return in_maps


def _make_consts():
    if "consts" not in _CACHE:
        tri = (np.arange(128)[None, :] >= np.arange(128)[:, None]).astype(np.float32)
        eye = np.eye(64, dtype=np.float32)
        c = np.zeros((128, 192), np.float32)
        c[:, 0:128] = tri
        c[0:64, 128:192] = eye
        c[64:128, 128:192] = eye
        _CACHE["consts"] = c.astype(BF16)
    return _CACHE["consts"]


def combine_partials(partials, bo):
    acc = np.zeros((D, BS), np.float32)
    for p in partials:
        acc += np.asarray(p, np.float32)
    out = acc.reshape(D, B, S).transpose(1, 2, 0) + np.asarray(bo, np.float32)[None, None, :]
    return np.ascontiguousarray(out.astype(np.float32))


def kernel(x, Wq, Wk, Wv, Wo, bo):
    from concourse.bass_utils import run_bass_kernel_spmd

    nc = get_nc()
    in_maps = make_in_maps(x, Wq, Wk, Wv, Wo)
    res = run_bass_kernel_spmd(nc, in_maps, core_ids=list(range(NCORES)))
    partials = [r["out_pT"] for r in res.results]
    return combine_partials(partials, bo)


# revision 4
# speedup vs baseline: 1.1595x; 1.1595x over previous
"""Multi-head causal attention (B=2, S=2048, D=1024, H=16) on 8 TRN2 cores.

Sharding: tensor-parallel over heads. Core c owns heads {2c, 2c+1} and rows
[128c, 128c+128) of Wo. Each core computes its heads' attention and the
partial output projection; the host sums the 8 partials (the "all-reduce")
and adds the bias.

Q/K projections run in fp8(e4m3) DoubleRow perf mode (K=256, M=128 per
matmul, 1.0 cycle/column = 2x bf16 FLOPs); x for the V projection streams
in group-major layout (xTg) through a small SBUF ring so V-filler matmuls
never wait on bulk DMA and SBUF fits the extra fp8 copy of x.

Device layout:
  xT8     [128, 4, 2, BS] fp8  x^T in d-chunk-pair layout for DoubleRow
  xTg     ring of [128, 8, 128] bf16 group tiles (V projection input)
  wq8/wk8 [128, 4, 2, 2, 64] fp8  pair-layout per local head
  wv      [128, 8, 128] bf16; wo [128, 1024] bf16
  qT/kT   [128, 4096] bf16; V_sb [128, 32, 2, 128] bf16; OT [128, 4096]

Per core:
  1. Q^T/K^T = fp8 DoubleRow (w.T @ xT), batch 0 inline, batch 1 as filler;
     V in [t, k] layout with a leading ones column (softmax denominators
     emitted by the attention matmul into PSUM partition 0).
  2. Causal attention in scores^T orientation, quarter-major deferred AV
     (see v1 docstring): exp stream on ACT, dense AV bursts, fast
     reciprocal + K=1 broadcast matmul normalize.
  3. Partial projection out_pT = wo_rows.T @ OT interleaved as PE filler.
"""

import numpy as np
import ml_dtypes

B, S, D, H = 2, 2048, 1024, 16
HD = 64          # head dim
NCORES = 8
HL = H // NCORES  # local heads per core = 2
BS = B * S        # 4096
SCALE = float(D) ** -0.5

BF16 = ml_dtypes.bfloat16
E4M3 = ml_dtypes.float8_e4m3fn

_CACHE = {}


def _build_kernel():
    import concourse.mybir as mybir
    import concourse.tile as tile
    from concourse import bacc

    bf16 = mybir.dt.bfloat16
    fp8 = mybir.dt.float8e4
    f32 = mybir.dt.float32
    Exp = mybir.ActivationFunctionType.Exp
    DR = mybir.MatmulPerfMode.DoubleRow

    nc = bacc.Bacc("TRN2", debug=False, enable_asserts=False)
    xT8_d = nc.dram_tensor("xT8", [128, 4, 2, BS], fp8, kind="ExternalInput").ap()
    xTg_d = nc.dram_tensor("xTg", [128, BS // 128, 8, 128], bf16, kind="ExternalInput").ap()
    wq8_d = nc.dram_tensor("wq8", [128, 4, 2, 128], fp8, kind="ExternalInput").ap()
    wk8_d = nc.dram_tensor("wk8", [128, 4, 2, 128], fp8, kind="ExternalInput").ap()
    wv_d = nc.dram_tensor("wv", [D, 128], bf16, kind="ExternalInput").ap()
    wo_d = nc.dram_tensor("wo", [128, D], bf16, kind="ExternalInput").ap()
    # consts: cols 0:128 = upper-tri mask (1 where col >= row), cols 128:192 =
    # 64x64 identity replicated in both partition halves.
    consts_d = nc.dram_tensor("consts", [128, 192], bf16, kind="ExternalInput").ap()
    out_d = nc.dram_tensor("out_pT", [D, BS], bf16, kind="ExternalOutput").ap()

    DC = D // 128   # 8 d-chunks
    OP = DC // 2    # 4 d-chunk pairs for DoubleRow
    NT = S // 128   # 16 key blocks per sequence
    NG = BS // 128  # 32 V groups

    with tile.TileContext(nc) as tc:
        with tc.tile_pool(name="persist", bufs=1) as pp:
            xT8 = pp.tile([128, OP, 2, BS], fp8, tag="xT8")
            qT = pp.tile([128, BS], bf16, tag="qT")
            kT = pp.tile([128, BS], bf16, tag="kT")
            # V in [t, k] layout, padded to 128 columns: col 0 = 1.0 (the
            # ones column makes the attention matmul emit softmax
            # denominators in PSUM partition 0), cols 1:64 = 0, cols
            # 64:128 = V block for s-block g and local head j.
            V_sb = pp.tile([128, NG, HL, 128], bf16, tag="V")
            OT = pp.tile([128, BS], bf16, tag="OT")
            wq8 = pp.tile([128, OP, 2, 128], fp8, tag="wq8")
            wk8 = pp.tile([128, OP, 2, 128], fp8, tag="wk8")
            wv = pp.tile([128, DC, 128], bf16, tag="wv")
            wo = pp.tile([128, D], bf16, tag="wo")
            consts = pp.tile([128, 192], bf16, tag="consts")
            trimask = consts[:, 0:128]
            ident = consts[:, 128:192]  # noqa: F841 (kept for layout parity)
            ones64 = pp.tile([1, 64], f32, tag="ones64")

            # DMA in. The phase-1 critical set (wk8, first xT8 columns, wq8)
            # issues from the ACT queue -- descriptor writes on the sync
            # engine are ~0.6us each and serialize startup otherwise. ACT's
            # exp warmup is emitted after these so it doesn't block them
            # (it waits on consts, which arrives via sync).
            nc.scalar.dma_start(wk8[:], wk8_d[:])
            for op in range(OP):
                nc.scalar.dma_start(xT8[:, op, :, 0:512], xT8_d[:, op, :, 0:512])
            nc.scalar.dma_start(wq8[:], wq8_d[:])
            for op in range(2):
                nc.scalar.dma_start(xT8[:, op, :, 512:2048], xT8_d[:, op, :, 512:2048])
            # sync queue: consts + V-projection inputs early (first V filler
            # runs right after attention starts), bulk batch-1 x later.
            nc.sync.dma_start(consts[:], consts_d[:])
            nc.sync.dma_start(wv[:], wv_d.rearrange("(o p) c -> p o c", p=128))
            for op in range(2, OP):
                nc.sync.dma_start(xT8[:, op, :, 512:2048], xT8_d[:, op, :, 512:2048])
            nc.vector.memset(ones64[:], 1.0)
            # Preheat the ACT exp table so the first real exp doesn't pay
            # the table-load latency mid-pipeline.
            warmup = pp.tile([1, 8], bf16, tag="warmup")
            nc.scalar.activation(warmup[:], consts[0:1, 0:8], Exp, scale=SCALE)

            # ---- Phase 1: Q^T / K^T fp8 DoubleRow projections, batch 0 ----
            # Batch 1's projection chunks are deferred into the attention
            # loop as filler (they're only needed from the third head on).
            # unit order: kT s0 first (first scores block needs it), then
            # qT s0..s3 (rhs of the first scores), then the rest of kT; each
            # unit copies to SBUF immediately so attention starts gap-free.
            with tc.tile_pool(name="ph1psum", bufs=8, space="PSUM") as ph1:
                units = [(wk8, kT, 0), (wq8, qT, 0), (wq8, qT, 1),
                         (wq8, qT, 2), (wq8, qT, 3), (wk8, kT, 1),
                         (wk8, kT, 2), (wk8, kT, 3)]
                for ui, (w_sb, dst, s) in enumerate(units):
                    ps1 = ph1.tile([128, 512], f32, tag="ph1", name="ph1")
                    for op in range(OP):
                        nc.tensor.matmul(
                            ps1[:],
                            lhsT=w_sb[:, op, :, :],
                            rhs=xT8[:, op, :, 512 * s : 512 * (s + 1)],
                            start=(op == 0),
                            stop=(op == OP - 1),
                            perf_mode=DR,
                        )
                    if ui % 2 == 0:
                        nc.scalar.copy(dst[:, 512 * s : 512 * (s + 1)], ps1[:])
                    else:
                        nc.vector.tensor_copy(dst[:, 512 * s : 512 * (s + 1)], ps1[:])

            # V_sb constants; emitted after phase-1 so these DVE ops
            # don't delay the urgent qT/kT copies.
            nc.vector.memset(V_sb[:, :, :, 0:HD], 0.0)
            nc.vector.memset(V_sb[:, :, :, 0:1], 1.0)

            # ---- Phases 2+3: V projection (streamed xTg), attention, and
            # output projection, all interleaved on the PE. ----
            with (
                tc.tile_pool(name="po", bufs=2, space="PSUM") as po_pool,
                tc.tile_pool(name="ps", bufs=2, space="PSUM") as ps_pool,
                tc.tile_pool(name="aux", bufs=2, space="PSUM") as aux_pool,
                tc.tile_pool(name="expp", bufs=26) as exp_pool,
                tc.tile_pool(name="xtg", bufs=8) as xtg_pool,
                tc.tile_pool(name="recip", bufs=3) as rc_pool,
                tc.tile_pool(name="onum", bufs=3) as on_pool,
                tc.tile_pool(name="ph4out", bufs=2) as ph4o,
            ):
                xtg_tiles = {}

                def fetch_group(g):
                    if g >= NG:
                        return
                    t = xtg_pool.tile([128, DC, 128], bf16, tag="xtg", name="xtg")
                    nc.sync.dma_start(t[:], xTg_d[:, g, :, :])
                    xtg_tiles[g] = t

                def emit_v(vps, g, tag="pv"):
                    # V directly in [t, k] layout: lhsT = xTg s-block
                    # (stationary), rhs = wv: psum[s, c] = V block for both
                    # local heads side by side.
                    fetch_group(g + 6)
                    src = xtg_tiles.pop(g)
                    pv = vps.tile([128, 128], f32, tag=tag, name="pv")
                    for o in range(DC):
                        nc.tensor.matmul(
                            pv[:],
                            lhsT=src[:, o, :],
                            rhs=wv[:, o, :],
                            start=(o == 0),
                            stop=(o == DC - 1),
                        )
                    nc.vector.tensor_copy(
                        V_sb[:, g, :, HD : 2 * HD],
                        pv[:].rearrange("p (j k) -> p j k", j=HL),
                    )

                # prefetch the first V groups before attention begins,
                # then queue the later-needed bulk inputs behind them
                for g in range(6):
                    fetch_group(g)
                nc.sync.dma_start(wo[:], wo_d[:])
                for op in range(OP):
                    nc.sync.dma_start(
                        xT8[:, op, :, 2048:4096], xT8_d[:, op, :, 2048:4096])

                # ---- causal attention per (b, local head) ----
                # Quarter-major deferred AV (see v1): scores/exp stream in kj
                # order, every exp tile kept in SBUF, each 512-wide o^T
                # quarter accumulated as one dense burst then normalized.
                def emit_block(b, j, kj, p0, p1):
                    qTh = qT[64 * j : 64 * (j + 1), S * b : S * (b + 1)]
                    kTh = kT[64 * j : 64 * (j + 1), S * b : S * (b + 1)]
                    s_lo = 128 * kj
                    w = p1 - p0
                    ps = ps_pool.tile([128, 1024], f32, tag="ps", name="ps")
                    for c0 in range(0, w, 512):
                        c1 = min(c0 + 512, w)
                        nc.tensor.matmul(
                            ps[:, c0:c1],
                            lhsT=kTh[:, s_lo : s_lo + 128],
                            rhs=qTh[:, p0 + c0 : p0 + c1],
                            start=True,
                            stop=True,
                        )
                    et = exp_pool.tile([128, 1024], bf16, tag="expT", name="et")
                    nc.scalar.activation(et[:, 0:w], ps[:, 0:w], Exp, scale=SCALE)
                    if p0 == s_lo:
                        # diagonal 128x128: keep only s' >= t
                        nc.vector.tensor_mul(et[:, 0:128], et[:, 0:128], trimask[:])
                    return et

                def emit_quarter(b, j, q, ets):
                    kj_last = 4 * q + 3
                    pq = po_pool.tile([128, 512], f32, tag="po", name="pq")
                    for k2 in range(kj_last + 1):
                        a0 = max(512 * q, 128 * k2)
                        a1 = 512 * (q + 1)
                        for p0, p1, et in ets[k2]:
                            if p0 <= a0 < p1:
                                break
                        else:
                            raise AssertionError("no piece")
                        nc.tensor.matmul(
                            pq[:, a0 - 512 * q : a1 - 512 * q],
                            lhsT=V_sb[:, NT * b + k2, j, :],
                            rhs=et[:, a0 - p0 : a1 - p0],
                            start=(k2 == 0),
                            stop=(k2 == kj_last),
                        )
                    return pq

                def emit_normalize(b, j, q, pq):
                    # o^T[k, s] / denom[s] for quarter q. Copy the numerator
                    # to SBUF (frees the po slot), fast-reciprocal the
                    # denominator row (PSUM partition 0), broadcast it
                    # across 64 partitions via a K=1 matmul, then a single
                    # one-PSUM-operand multiply into OT.
                    onum = on_pool.tile([64, 512], f32, tag="onum", name="onum")
                    nc.scalar.copy(onum[:], pq[HD : 2 * HD, :])
                    rc = rc_pool.tile([1, 512], f32, tag="rc", name="rc")
                    nc.vector.reciprocal_approx_fast(rc[:], pq[0:1, :])
                    pb = aux_pool.tile([64, 512], f32, tag="aux", name="pb")
                    nc.tensor.matmul(pb[:], lhsT=ones64[:], rhs=rc[:], start=True, stop=True)
                    nc.vector.tensor_mul(
                        OT[64 * j : 64 * (j + 1),
                           S * b + 512 * q : S * b + 512 * (q + 1)],
                        onum[:],
                        pb[:],
                    )

                out_r = out_d.rearrange("(o p) s -> p o s", p=128)

                def emit_ph4_nb(b, nb, rush=False):
                    # partial projection for 512 columns of batch b:
                    # out_pT[:, cols] = wo.T @ OT[:, cols]. rush=True (the
                    # kernel's last chunk): two half-stages with the casts
                    # alternating ACT/DVE so the tail isn't serialized on
                    # one engine.
                    for half in range(2 if rush else 1):
                        dcs = range(half * 4, half * 4 + 4) if rush else range(DC)
                        stage = ph4o.tile([128, DC, 512], bf16, tag="o4", name="o4")
                        for i, dc in enumerate(dcs):
                            pp4 = aux_pool.tile([128, 512], f32, tag="aux", name="pp4")
                            nc.tensor.matmul(
                                pp4[:],
                                lhsT=wo[:, 128 * dc : 128 * (dc + 1)],
                                rhs=OT[:, S * b + 512 * nb : S * b + 512 * (nb + 1)],
                                start=True,
                                stop=True,
                            )
                            if rush and i % 2 == 0:
                                nc.scalar.copy(stage[:, dc, :], pp4[:])
                            else:
                                nc.vector.tensor_copy(stage[:, dc, :], pp4[:])
                        rows = (slice(half * 4, half * 4 + 4) if rush
                                else slice(0, DC))
                        nc.sync.dma_start(
                            out_r[:, rows, S * b + 512 * nb : S * b + 512 * (nb + 1)],
                            stage[:, rows, :],
                        )

                def emit_qk1(w_sb, dst, sc):
                    # one batch-1 projection chunk: 8 fp8 DoubleRow matmuls
                    pk = aux_pool.tile([128, 512], f32, tag="aux", name="pk")
                    for op in range(OP):
                        nc.tensor.matmul(
                            pk[:],
                            lhsT=w_sb[:, op, :, :],
                            rhs=xT8[:, op, :, 512 * sc : 512 * (sc + 1)],
                            start=(op == 0),
                            stop=(op == OP - 1),
                            perf_mode=DR,
                        )
                    nc.vector.tensor_copy(dst[:, 512 * sc : 512 * (sc + 1)], pk[:])

                filler_q = [("v", g) for g in range(0, BS // 256)]
                for sc in range(4, 8):
                    filler_q.append(("qk", wq8, qT, sc))
                    filler_q.append(("qk", wk8, kT, sc))
                filler_q += [("v", g) for g in range(BS // 256, BS // 128)]

                def emit_filler():
                    if not filler_q:
                        return
                    u = filler_q.pop(0)
                    if u[0] == "qk":
                        emit_qk1(u[1], u[2], u[3])
                    else:
                        emit_v(aux_pool, u[1], tag="aux")

                ph4_queue = []      # (b, nb) chunks awaiting emission
                ph4_state = None    # (b, nb, stage, next_dc)

                def emit_ph4_step():
                    # one dc-chunk of a pending output-projection unit
                    nonlocal ph4_state
                    if ph4_state is None:
                        if not ph4_queue:
                            return
                        b4, nb4 = ph4_queue.pop(0)
                        stage = ph4o.tile([128, DC, 512], bf16, tag="o4", name="o4")
                        ph4_state = (b4, nb4, stage, 0)
                    b4, nb4, stage, dc = ph4_state
                    pp4 = aux_pool.tile([128, 512], f32, tag="aux", name="pp4")
                    nc.tensor.matmul(
                        pp4[:],
                        lhsT=wo[:, 128 * dc : 128 * (dc + 1)],
                        rhs=OT[:, S * b4 + 512 * nb4 : S * b4 + 512 * (nb4 + 1)],
                        start=True,
                        stop=True,
                    )
                    nc.vector.tensor_copy(stage[:, dc, :], pp4[:])
                    if dc == DC - 1:
                        nc.sync.dma_start(
                            out_r[:, :, S * b4 + 512 * nb4 : S * b4 + 512 * (nb4 + 1)],
                            stage[:],
                        )
                        ph4_state = None
                    else:
                        ph4_state = (b4, nb4, stage, dc + 1)

                for bh in range(B * HL):
                    b, j = bh // HL, bh % HL
                    ets = {}
                    for kj in range(NT):
                        s_lo = 128 * kj
                        pieces = ([(s_lo, 1024), (1024, 2048)] if s_lo < 1024
                                  else [(s_lo, 2048)])
                        ets[kj] = [(p0, p1, emit_block(b, j, kj, p0, p1))
                                   for p0, p1 in pieces]
                        # quarter q's AV burst is deferred by one block so
                        # its final exp tile is ready when the burst reaches
                        # it (quarter 3 naturally gets this slack from the
                        # next head's first blocks)
                        if kj % 4 == 0 and kj > 0:
                            q = kj // 4 - 1
                            pq = emit_quarter(b, j, q, ets)
                            emit_normalize(b, j, q, pq)
                            if bh == 3:
                                ph4_queue.append((1, q))
                        if bh == 2 and kj % 4 == 0 and kj > 0:
                            # batch-0 output projection as bh2 filler (OT
                            # batch-0 columns completed during bh1),
                            # spread one dc-chunk per kj step below
                            ph4_queue.append((0, kj // 4 - 1))
                        if bh >= 2:
                            emit_ph4_step()
                            emit_ph4_step()
                        if bh <= 1:
                            # one filler unit per kj in bh0 (paced to the V
                            # deadlines), two per kj in bh1 to drain
                            emit_filler()
                            if bh == 1:
                                emit_filler()
                    q = 3
                    pq = emit_quarter(b, j, q, ets)
                    emit_normalize(b, j, q, pq)
                    if bh == 1:
                        # anything batch-1 still pending must land before bh2
                        while filler_q:
                            emit_filler()
                    if bh == 2:
                        ph4_queue.append((0, 3))
                    if bh == 3:
                        while ph4_queue or ph4_state is not None:
                            emit_ph4_step()
                        emit_ph4_nb(1, 3, rush=True)

    nc.compile()
    return nc


def get_nc():
    if "nc" not in _CACHE:
        _CACHE["nc"] = _build_kernel()
    return _CACHE["nc"]


def make_in_maps(x, Wq, Wk, Wv, Wo):
    """Host-side sharding: per-core input dict (numpy)."""
    x = np.asarray(x, np.float32)
    Wq = np.asarray(Wq, np.float32)
    Wk = np.asarray(Wk, np.float32)
    Wv = np.asarray(Wv, np.float32)
    Wo = np.asarray(Wo, np.float32)
    xT = np.ascontiguousarray(x.transpose(2, 0, 1).reshape(D, BS))
    # x quantized to bf16 first (the v1 baseline numerics), then to e4m3 for
    # the fp8 Q/K path so device and host casts agree.
    xTb = xT.astype(BF16)
    # xT8[p, op, i, s] = xT[128*(2*op+i)+p, s]
    xT8 = np.ascontiguousarray(
        xTb.astype(np.float32).reshape(4, 2, 128, BS).transpose(2, 0, 1, 3)
    ).astype(E4M3)
    # xTg[p, g, o, c] = xT[128*o+p, 128*g+c]
    xTg = np.ascontiguousarray(
        xTb.reshape(D // 128, 128, BS // 128, 128).transpose(1, 2, 0, 3)
    )

    in_maps = []
    for c in range(NCORES):
        h0 = HL * c

        def pack8(W):
            # [HL, D, HD] -> [128, 4, 2, 128] e4m3 pair layout, both heads
            # packed in the last dim (j*64+m)
            Wl = W[h0 : h0 + HL].astype(BF16).astype(np.float32)
            return np.ascontiguousarray(
                Wl.reshape(HL, 4, 2, 128, HD).transpose(3, 1, 2, 0, 4)
                .reshape(128, 4, 2, 128)
            ).astype(E4M3)

        wv_l = np.ascontiguousarray(
            Wv[h0 : h0 + HL].transpose(1, 0, 2).reshape(D, HL * HD)
        ).astype(BF16)
        in_maps.append(
            {
                "xT8": xT8,
                "xTg": xTg,
                "wq8": pack8(Wq),
                "wk8": pack8(Wk),
                "wv": wv_l,
                "wo": np.ascontiguousarray(Wo[128 * c : 128 * (c + 1), :]).astype(BF16),
                "consts": _make_consts(),
            }
        )
    return in_maps


def _make_consts():
    if "consts" not in _CACHE:
        tri = (np.arange(128)[None, :] >= np.arange(128)[:, None]).astype(np.float32)
        eye = np.eye(64, dtype=np.float32)
        c = np.zeros((128, 192), np.float32)
        c[:, 0:128] = tri
        c[0:64, 128:192] = eye
        c[64:128, 128:192] = eye
        _CACHE["consts"] = c.astype(BF16)
    return _CACHE["consts"]


def combine_partials(partials, bo):
    acc = np.zeros((D, BS), np.float32)
    for p in partials:
        acc += np.asarray(p, np.float32)
    out = acc.reshape(D, B, S).transpose(1, 2, 0) + np.asarray(bo, np.float32)[None, None, :]
    return np.ascontiguousarray(out.astype(np.float32))


def kernel(x, Wq, Wk, Wv, Wo, bo):
    from concourse.bass_utils import run_bass_kernel_spmd

    nc = get_nc()
    in_maps = make_in_maps(x, Wq, Wk, Wv, Wo)
    res = run_bass_kernel_spmd(nc, in_maps, core_ids=list(range(NCORES)))
    partials = [r["out_pT"] for r in res.results]
    return combine_partials(partials, bo)


# revision 5
# speedup vs baseline: 1.1987x; 1.0338x over previous
"""Multi-head causal attention (B=2, S=2048, D=1024, H=16) on 8 TRN2 cores.

Sharding: tensor-parallel over heads. Core c owns heads {2c, 2c+1} and rows
[128c, 128c+128) of Wo. Each core computes its heads' attention and the
partial output projection; the host sums the 8 partials (the "all-reduce")
and adds the bias.

Q/K projections run in fp8(e4m3) DoubleRow perf mode (K=256, M=128 per
matmul, 1.0 cycle/column = 2x bf16 FLOPs); x for the V projection streams
in group-major layout (xTg) through a small SBUF ring so V-filler matmuls
never wait on bulk DMA and SBUF fits the extra fp8 copy of x.

Device layout:
  xT8     [128, 4, 2, BS] fp8  x^T in d-chunk-pair layout for DoubleRow
  xTg     ring of [128, 8, 128] bf16 group tiles (V projection input)
  wq8/wk8 [128, 4, 2, 2, 64] fp8  pair-layout per local head
  wv      [128, 8, 128] bf16; wo [128, 1024] bf16
  qT/kT   [128, 4096] bf16; V_sb [128, 32, 2, 128] bf16; OT [128, 4096]

Per core:
  1. Q^T/K^T = fp8 DoubleRow (w.T @ xT), batch 0 inline, batch 1 as filler;
     V in [t, k] layout with a leading ones column (softmax denominators
     emitted by the attention matmul into PSUM partition 0).
  2. Causal attention in scores^T orientation, quarter-major deferred AV
     (see v1 docstring): exp stream on ACT, dense AV bursts, fast
     reciprocal + K=1 broadcast matmul normalize.
  3. Partial projection out_pT = wo_rows.T @ OT interleaved as PE filler.
"""

import numpy as np
import ml_dtypes

B, S, D, H = 2, 2048, 1024, 16
HD = 64          # head dim
NCORES = 8
HL = H // NCORES  # local heads per core = 2
BS = B * S        # 4096
SCALE = float(D) ** -0.5

BF16 = ml_dtypes.bfloat16
E4M3 = ml_dtypes.float8_e4m3fn

_CACHE = {}


def _build_kernel():
    import concourse.mybir as mybir
    import concourse.tile as tile
    from concourse import bacc

    bf16 = mybir.dt.bfloat16
    fp8 = mybir.dt.float8e4
    f32 = mybir.dt.float32
    Exp = mybir.ActivationFunctionType.Exp
    DR = mybir.MatmulPerfMode.DoubleRow

    nc = bacc.Bacc("TRN2", debug=False, enable_asserts=False)
    xT8_d = nc.dram_tensor("xT8", [128, 4, 2, BS], fp8, kind="ExternalInput").ap()
    xTg_d = nc.dram_tensor("xTg", [128, BS // 128, 8, 128], bf16, kind="ExternalInput").ap()
    wq8_d = nc.dram_tensor("wq8", [128, 4, 2, 128], fp8, kind="ExternalInput").ap()
    wk8_d = nc.dram_tensor("wk8", [128, 4, 2, 128], fp8, kind="ExternalInput").ap()
    wv_d = nc.dram_tensor("wv", [D, 128], bf16, kind="ExternalInput").ap()
    wo_d = nc.dram_tensor("wo", [128, D], bf16, kind="ExternalInput").ap()
    # consts: cols 0:128 = upper-tri mask (1 where col >= row), cols 128:192 =
    # 64x64 identity replicated in both partition halves.
    consts_d = nc.dram_tensor("consts", [128, 192], bf16, kind="ExternalInput").ap()
    out_d = nc.dram_tensor("out_pT", [D, BS], bf16, kind="ExternalOutput").ap()

    DC = D // 128   # 8 d-chunks
    OP = DC // 2    # 4 d-chunk pairs for DoubleRow
    NT = S // 128   # 16 key blocks per sequence
    NG = BS // 128  # 32 V groups

    with tile.TileContext(nc) as tc:
        with tc.tile_pool(name="persist", bufs=1) as pp:
            xT8 = pp.tile([128, OP, 2, BS], fp8, tag="xT8")
            qT = pp.tile([128, BS], bf16, tag="qT")
            kT = pp.tile([128, BS], bf16, tag="kT")
            # V in [t, k] layout, padded to 128 columns: col 0 = 1.0 (the
            # ones column makes the attention matmul emit softmax
            # denominators in PSUM partition 0), cols 1:64 = 0, cols
            # 64:128 = V block for s-block g and local head j.
            V_sb = pp.tile([128, NG, HL, 128], bf16, tag="V")
            OT = pp.tile([128, BS], bf16, tag="OT")
            wq8 = pp.tile([128, OP, 2, 128], fp8, tag="wq8")
            wk8 = pp.tile([128, OP, 2, 128], fp8, tag="wk8")
            wv = pp.tile([128, DC, 128], bf16, tag="wv")
            wo = pp.tile([128, D], bf16, tag="wo")
            consts = pp.tile([128, 192], bf16, tag="consts")
            trimask = consts[:, 0:128]
            ident = consts[:, 128:192]  # noqa: F841 (kept for layout parity)
            ones64 = pp.tile([1, 64], f32, tag="ones64")

            # DMA in. The phase-1 critical set (wk8, first xT8 columns, wq8)
            # issues from the ACT queue -- descriptor writes on the sync
            # engine are ~0.6us each and serialize startup otherwise. ACT's
            # exp warmup is emitted after these so it doesn't block them
            # (it waits on consts, which arrives via sync).
            nc.scalar.dma_start(wk8[:], wk8_d[:])
            for op in range(OP):
                nc.scalar.dma_start(xT8[:, op, :, 0:512], xT8_d[:, op, :, 0:512])
            nc.scalar.dma_start(wq8[:], wq8_d[:])
            for op in range(2):
                nc.scalar.dma_start(xT8[:, op, :, 512:2048], xT8_d[:, op, :, 512:2048])
            # sync queue: consts + V-projection inputs early (first V filler
            # runs right after attention starts), bulk batch-1 x later.
            nc.sync.dma_start(consts[:], consts_d[:])
            nc.sync.dma_start(wv[:], wv_d.rearrange("(o p) c -> p o c", p=128))
            for op in range(2, OP):
                nc.sync.dma_start(xT8[:, op, :, 512:2048], xT8_d[:, op, :, 512:2048])
            nc.vector.memset(ones64[:], 1.0)
            # Preheat the ACT exp table so the first real exp doesn't pay
            # the table-load latency mid-pipeline.
            warmup = pp.tile([1, 8], bf16, tag="warmup")
            nc.scalar.activation(warmup[:], consts[0:1, 0:8], Exp, scale=SCALE)

            # ---- Phase 1: Q^T / K^T fp8 DoubleRow projections, batch 0 ----
            # Batch 1's projection chunks are deferred into the attention
            # loop as filler (they're only needed from the third head on).
            # unit order: kT s0 first (first scores block needs it), then
            # qT s0..s3 (rhs of the first scores), then the rest of kT; each
            # unit copies to SBUF immediately so attention starts gap-free.
            with tc.tile_pool(name="ph1psum", bufs=8, space="PSUM") as ph1:
                units = [(wk8, kT, 0), (wq8, qT, 0), (wq8, qT, 1),
                         (wq8, qT, 2), (wq8, qT, 3), (wk8, kT, 1),
                         (wk8, kT, 2), (wk8, kT, 3)]
                for ui, (w_sb, dst, s) in enumerate(units):
                    ps1 = ph1.tile([128, 512], f32, tag="ph1", name="ph1")
                    for op in range(OP):
                        nc.tensor.matmul(
                            ps1[:],
                            lhsT=w_sb[:, op, :, :],
                            rhs=xT8[:, op, :, 512 * s : 512 * (s + 1)],
                            start=(op == 0),
                            stop=(op == OP - 1),
                            perf_mode=DR,
                        )
                    if ui % 2 == 0:
                        nc.scalar.copy(dst[:, 512 * s : 512 * (s + 1)], ps1[:])
                    else:
                        nc.vector.tensor_copy(dst[:, 512 * s : 512 * (s + 1)], ps1[:])

            # V_sb constants; emitted after phase-1 so these DVE ops
            # don't delay the urgent qT/kT copies.
            nc.vector.memset(V_sb[:, :, :, 0:HD], 0.0)
            nc.vector.memset(V_sb[:, :, :, 0:1], 1.0)

            # ---- Phases 2+3: V projection (streamed xTg), attention, and
            # output projection, all interleaved on the PE. ----
            with (
                tc.tile_pool(name="po", bufs=2, space="PSUM") as po_pool,
                tc.tile_pool(name="ps", bufs=2, space="PSUM") as ps_pool,
                tc.tile_pool(name="aux", bufs=2, space="PSUM") as aux_pool,
                tc.tile_pool(name="expp", bufs=26) as exp_pool,
                tc.tile_pool(name="xtg", bufs=8) as xtg_pool,
                tc.tile_pool(name="recip", bufs=3) as rc_pool,
                tc.tile_pool(name="onum", bufs=3) as on_pool,
                tc.tile_pool(name="ph4out", bufs=2) as ph4o,
            ):
                xtg_tiles = {}

                def fetch_group(g):
                    if g >= NG:
                        return
                    t = xtg_pool.tile([128, DC, 128], bf16, tag="xtg", name="xtg")
                    nc.sync.dma_start(t[:], xTg_d[:, g, :, :])
                    xtg_tiles[g] = t

                def emit_v(vps, g, tag="pv"):
                    # V directly in [t, k] layout: lhsT = xTg s-block
                    # (stationary), rhs = wv: psum[s, c] = V block for both
                    # local heads side by side.
                    fetch_group(g + 6)
                    src = xtg_tiles.pop(g)
                    pv = vps.tile([128, 128], f32, tag=tag, name="pv")
                    for o in range(DC):
                        nc.tensor.matmul(
                            pv[:],
                            lhsT=src[:, o, :],
                            rhs=wv[:, o, :],
                            start=(o == 0),
                            stop=(o == DC - 1),
                        )
                    nc.vector.tensor_copy(
                        V_sb[:, g, :, HD : 2 * HD],
                        pv[:].rearrange("p (j k) -> p j k", j=HL),
                    )

                # prefetch the first V groups before attention begins;
                # batch-1 x and wo are issued mid-bh0 (below) so they don't
                # starve the xTg group stream on the sync queue
                for g in range(6):
                    fetch_group(g)

                # ---- causal attention per (b, local head) ----
                # Quarter-major deferred AV (see v1): scores/exp stream in kj
                # order, every exp tile kept in SBUF, each 512-wide o^T
                # quarter accumulated as one dense burst then normalized.
                def emit_block(b, j, kj, p0, p1):
                    qTh = qT[64 * j : 64 * (j + 1), S * b : S * (b + 1)]
                    kTh = kT[64 * j : 64 * (j + 1), S * b : S * (b + 1)]
                    s_lo = 128 * kj
                    w = p1 - p0
                    ps = ps_pool.tile([128, 1024], f32, tag="ps", name="ps")
                    for c0 in range(0, w, 512):
                        c1 = min(c0 + 512, w)
                        nc.tensor.matmul(
                            ps[:, c0:c1],
                            lhsT=kTh[:, s_lo : s_lo + 128],
                            rhs=qTh[:, p0 + c0 : p0 + c1],
                            start=True,
                            stop=True,
                        )
                    et = exp_pool.tile([128, 1024], bf16, tag="expT", name="et")
                    nc.scalar.activation(et[:, 0:w], ps[:, 0:w], Exp, scale=SCALE)
                    if p0 == s_lo:
                        # diagonal 128x128: keep only s' >= t
                        nc.vector.tensor_mul(et[:, 0:128], et[:, 0:128], trimask[:])
                    return et

                def emit_quarter(b, j, q, ets):
                    kj_last = 4 * q + 3
                    pq = po_pool.tile([128, 512], f32, tag="po", name="pq")
                    for k2 in range(kj_last + 1):
                        a0 = max(512 * q, 128 * k2)
                        a1 = 512 * (q + 1)
                        for p0, p1, et in ets[k2]:
                            if p0 <= a0 < p1:
                                break
                        else:
                            raise AssertionError("no piece")
                        nc.tensor.matmul(
                            pq[:, a0 - 512 * q : a1 - 512 * q],
                            lhsT=V_sb[:, NT * b + k2, j, :],
                            rhs=et[:, a0 - p0 : a1 - p0],
                            start=(k2 == 0),
                            stop=(k2 == kj_last),
                        )
                    return pq

                def emit_normalize(b, j, q, pq):
                    # o^T[k, s] / denom[s] for quarter q. Copy the numerator
                    # to SBUF (frees the po slot), fast-reciprocal the
                    # denominator row (PSUM partition 0), broadcast it
                    # across 64 partitions via a K=1 matmul, then a single
                    # one-PSUM-operand multiply into OT.
                    onum = on_pool.tile([64, 512], f32, tag="onum", name="onum")
                    nc.scalar.copy(onum[:], pq[HD : 2 * HD, :])
                    rc = rc_pool.tile([1, 512], f32, tag="rc", name="rc")
                    nc.vector.reciprocal_approx_fast(rc[:], pq[0:1, :])
                    pb = aux_pool.tile([64, 512], f32, tag="aux", name="pb")
                    nc.tensor.matmul(pb[:], lhsT=ones64[:], rhs=rc[:], start=True, stop=True)
                    nc.vector.tensor_mul(
                        OT[64 * j : 64 * (j + 1),
                           S * b + 512 * q : S * b + 512 * (q + 1)],
                        onum[:],
                        pb[:],
                    )

                out_r = out_d.rearrange("(o p) s -> p o s", p=128)

                def emit_ph4_nb(b, nb, rush=False):
                    # partial projection for 512 columns of batch b:
                    # out_pT[:, cols] = wo.T @ OT[:, cols]. rush=True (the
                    # kernel's last chunk): two half-stages with the casts
                    # alternating ACT/DVE so the tail isn't serialized on
                    # one engine.
                    for half in range(2 if rush else 1):
                        dcs = range(half * 4, half * 4 + 4) if rush else range(DC)
                        stage = ph4o.tile([128, DC, 512], bf16, tag="o4", name="o4")
                        for i, dc in enumerate(dcs):
                            pp4 = aux_pool.tile([128, 512], f32, tag="aux", name="pp4")
                            nc.tensor.matmul(
                                pp4[:],
                                lhsT=wo[:, 128 * dc : 128 * (dc + 1)],
                                rhs=OT[:, S * b + 512 * nb : S * b + 512 * (nb + 1)],
                                start=True,
                                stop=True,
                            )
                            if rush and i % 2 == 0:
                                nc.scalar.copy(stage[:, dc, :], pp4[:])
                            else:
                                nc.vector.tensor_copy(stage[:, dc, :], pp4[:])
                        rows = (slice(half * 4, half * 4 + 4) if rush
                                else slice(0, DC))
                        nc.sync.dma_start(
                            out_r[:, rows, S * b + 512 * nb : S * b + 512 * (nb + 1)],
                            stage[:, rows, :],
                        )

                def emit_qk1(w_sb, dst, sc):
                    # one batch-1 projection chunk: 8 fp8 DoubleRow matmuls
                    pk = aux_pool.tile([128, 512], f32, tag="aux", name="pk")
                    for op in range(OP):
                        nc.tensor.matmul(
                            pk[:],
                            lhsT=w_sb[:, op, :, :],
                            rhs=xT8[:, op, :, 512 * sc : 512 * (sc + 1)],
                            start=(op == 0),
                            stop=(op == OP - 1),
                            perf_mode=DR,
                        )
                    nc.vector.tensor_copy(dst[:, 512 * sc : 512 * (sc + 1)], pk[:])

                filler_q = [("v", g) for g in range(0, BS // 256)]
                for sc in range(4, 8):
                    filler_q.append(("qk", wq8, qT, sc))
                    filler_q.append(("qk", wk8, kT, sc))
                filler_q += [("v", g) for g in range(BS // 256, BS // 128)]

                def emit_filler():
                    if not filler_q:
                        return
                    u = filler_q.pop(0)
                    if u[0] == "qk":
                        emit_qk1(u[1], u[2], u[3])
                    else:
                        emit_v(aux_pool, u[1], tag="aux")

                ph4_queue = []      # (b, nb) chunks awaiting emission
                ph4_state = None    # (b, nb, stage, next_dc)

                def emit_ph4_step():
                    # one dc-chunk of a pending output-projection unit
                    nonlocal ph4_state
                    if ph4_state is None:
                        if not ph4_queue:
                            return
                        b4, nb4 = ph4_queue.pop(0)
                        stage = ph4o.tile([128, DC, 512], bf16, tag="o4", name="o4")
                        ph4_state = (b4, nb4, stage, 0)
                    b4, nb4, stage, dc = ph4_state
                    pp4 = aux_pool.tile([128, 512], f32, tag="aux", name="pp4")
                    nc.tensor.matmul(
                        pp4[:],
                        lhsT=wo[:, 128 * dc : 128 * (dc + 1)],
                        rhs=OT[:, S * b4 + 512 * nb4 : S * b4 + 512 * (nb4 + 1)],
                        start=True,
                        stop=True,
                    )
                    nc.vector.tensor_copy(stage[:, dc, :], pp4[:])
                    if dc == DC - 1:
                        nc.sync.dma_start(
                            out_r[:, :, S * b4 + 512 * nb4 : S * b4 + 512 * (nb4 + 1)],
                            stage[:],
                        )
                        ph4_state = None
                    else:
                        ph4_state = (b4, nb4, stage, dc + 1)

                for bh in range(B * HL):
                    b, j = bh // HL, bh % HL
                    ets = {}
                    for kj in range(NT):
                        s_lo = 128 * kj
                        pieces = ([(s_lo, 1024), (1024, 2048)] if s_lo < 1024
                                  else [(s_lo, 2048)])
                        ets[kj] = [(p0, p1, emit_block(b, j, kj, p0, p1))
                                   for p0, p1 in pieces]
                        # quarter q's AV burst is deferred by one block so
                        # its final exp tile is ready when the burst reaches
                        # it (quarter 3 naturally gets this slack from the
                        # next head's first blocks)
                        if kj % 4 == 0 and kj > 0:
                            q = kj // 4 - 1
                            pq = emit_quarter(b, j, q, ets)
                            emit_normalize(b, j, q, pq)
                            if bh == 3:
                                ph4_queue.append((1, q))
                        if bh == 0 and kj == 6:
                            # batch-1 x for the qk1 fillers (needed from
                            # bh1); bh0's group prefetches got the head
                            # start on the sync queue
                            for op in range(OP):
                                nc.sync.dma_start(
                                    xT8[:, op, :, 2048:4096],
                                    xT8_d[:, op, :, 2048:4096])
                        if bh == 0 and kj == 10:
                            nc.sync.dma_start(wo[:], wo_d[:])
                        if bh == 2 and kj % 4 == 0 and kj > 0:
                            # batch-0 output projection as bh2 filler (OT
                            # batch-0 columns completed during bh1),
                            # spread one dc-chunk per kj step below
                            ph4_queue.append((0, kj // 4 - 1))
                        if bh >= 2:
                            emit_ph4_step()
                            emit_ph4_step()
                        if bh <= 1:
                            # one filler unit per kj in bh0 (paced to the V
                            # deadlines), two per kj in bh1 to drain
                            emit_filler()
                            if bh == 1:
                                emit_filler()
                    q = 3
                    pq = emit_quarter(b, j, q, ets)
                    emit_normalize(b, j, q, pq)
                    if bh == 1:
                        # anything batch-1 still pending must land before bh2
                        while filler_q:
                            emit_filler()
                    if bh == 2:
                        ph4_queue.append((0, 3))
                    if bh == 3:
                        while ph4_queue or ph4_state is not None:
                            emit_ph4_step()
                        emit_ph4_nb(1, 3, rush=True)

    nc.compile()
    return nc


def get_nc():
    if "nc" not in _CACHE:
        _CACHE["nc"] = _build_kernel()
    return _CACHE["nc"]


def make_in_maps(x, Wq, Wk, Wv, Wo):
    """Host-side sharding: per-core input dict (numpy)."""
    x = np.asarray(x, np.float32)
    Wq = np.asarray(Wq, np.float32)
    Wk = np.asarray(Wk, np.float32)
    Wv = np.asarray(Wv, np.float32)
    Wo = np.asarray(Wo, np.float32)
    xT = np.ascontiguousarray(x.transpose(2, 0, 1).reshape(D, BS))
    # x quantized to bf16 first (the v1 baseline numerics), then to e4m3 for
    # the fp8 Q/K path so device and host casts agree.
    xTb = xT.astype(BF16)
    # xT8[p, op, i, s] = xT[128*(2*op+i)+p, s]
    xT8 = np.ascontiguousarray(
        xTb.astype(np.float32).reshape(4, 2, 128, BS).transpose(2, 0, 1, 3)
    ).astype(E4M3)
    # xTg[p, g, o, c] = xT[128*o+p, 128*g+c]
    xTg = np.ascontiguousarray(
        xTb.reshape(D // 128, 128, BS // 128, 128).transpose(1, 2, 0, 3)
    )

    in_maps = []
    for c in range(NCORES):
        h0 = HL * c

        def pack8(W):
            # [HL, D, HD] -> [128, 4, 2, 128] e4m3 pair layout, both heads
            # packed in the last dim (j*64+m)
            Wl = W[h0 : h0 + HL].astype(BF16).astype(np.float32)
            return np.ascontiguousarray(
                Wl.reshape(HL, 4, 2, 128, HD).transpose(3, 1, 2, 0, 4)
                .reshape(128, 4, 2, 128)
            ).astype(E4M3)

        wv_l = np.ascontiguousarray(
            Wv[h0 : h0 + HL].transpose(1, 0, 2).reshape(D, HL * HD)
        ).astype(BF16)
        in_maps.append(
            {
                "xT8": xT8,
                "xTg": xTg,
                "wq8": pack8(Wq),
                "wk8": pack8(Wk),
                "wv": wv_l,
                "wo": np.ascontiguousarray(Wo[128 * c : 128 * (c + 1), :]).astype(BF16),
                "consts": _make_consts(),
            }
        )
    return in_maps


def _make_consts():
    if "consts" not in _CACHE:
        tri = (np.arange(128)[None, :] >= np.arange(128)[:, None]).astype(np.float32)
        eye = np.eye(64, dtype=np.float32)
        c = np.zeros((128, 192), np.float32)
        c[:, 0:128] = tri
        c[0:64, 128:192] = eye
        c[64:128, 128:192] = eye
        _CACHE["consts"] = c.astype(BF16)
    return _CACHE["consts"]


def combine_partials(partials, bo):
    acc = np.zeros((D, BS), np.float32)
    for p in partials:
        acc += np.asarray(p, np.float32)
    out = acc.reshape(D, B, S).transpose(1, 2, 0) + np.asarray(bo, np.float32)[None, None, :]
    return np.ascontiguousarray(out.astype(np.float32))


def kernel(x, Wq, Wk, Wv, Wo, bo):
    from concourse.bass_utils import run_bass_kernel_spmd

    nc = get_nc()
    in_maps = make_in_maps(x, Wq, Wk, Wv, Wo)
    res = run_bass_kernel_spmd(nc, in_maps, core_ids=list(range(NCORES)))
    partials = [r["out_pT"] for r in res.results]
    return combine_partials(partials, bo)


# revision 6
# speedup vs baseline: 1.3543x; 1.1298x over previous
"""Multi-head causal attention (B=2, S=2048, D=1024, H=16) on 8 TRN2 cores.

Sharding: tensor-parallel over heads. Core c owns heads {2c, 2c+1} and rows
[128c, 128c+128) of Wo. Each core computes its heads' attention and the
partial output projection; the host sums the 8 partials (the "all-reduce")
and adds the bias.

Q/K projections run in fp8(e4m3) DoubleRow perf mode (K=256, M=128 per
matmul, 1.0 cycle/column = 2x bf16 FLOPs); x for the V projection streams
in group-major layout (xTg) through a small SBUF ring so V-filler matmuls
never wait on bulk DMA and SBUF fits the extra fp8 copy of x.

Device layout:
  xT8     [128, 4, 2, BS] fp8  x^T in d-chunk-pair layout for DoubleRow
  xTg     ring of [128, 8, 128] bf16 group tiles (V projection input)
  wq8/wk8 [128, 4, 2, 2, 64] fp8  pair-layout per local head
  wv      [128, 8, 128] bf16; wo [128, 1024] bf16
  qT/kT   [128, 4096] bf16; V_sb [128, 32, 2, 128] bf16; OT [128, 4096]

Per core:
  1. Q^T/K^T = fp8 DoubleRow (w.T @ xT), batch 0 inline, batch 1 as filler;
     V in [t, k] layout with a leading ones column (softmax denominators
     emitted by the attention matmul into PSUM partition 0).
  2. Causal attention in scores^T orientation, quarter-major deferred AV
     (see v1 docstring): exp stream on ACT, dense AV bursts, fast
     reciprocal + K=1 broadcast matmul normalize.
  3. Partial projection out_pT = wo_rows.T @ OT interleaved as PE filler.
"""

import numpy as np
import ml_dtypes

B, S, D, H = 2, 2048, 1024, 16
HD = 64          # head dim
NCORES = 8
HL = H // NCORES  # local heads per core = 2
BS = B * S        # 4096
SCALE = float(D) ** -0.5

BF16 = ml_dtypes.bfloat16
E4M3 = ml_dtypes.float8_e4m3fn

_CACHE = {}


def _build_kernel():
    import concourse.mybir as mybir
    import concourse.tile as tile
    from concourse import bacc

    bf16 = mybir.dt.bfloat16
    fp8 = mybir.dt.float8e4
    f32 = mybir.dt.float32
    Exp = mybir.ActivationFunctionType.Exp
    DR = mybir.MatmulPerfMode.DoubleRow

    nc = bacc.Bacc("TRN2", debug=False, enable_asserts=False)
    xT8_d = nc.dram_tensor("xT8", [128, 4, 2, BS], fp8, kind="ExternalInput").ap()
    xTg_d = nc.dram_tensor("xTg", [128, BS // 128, 8, 128], bf16, kind="ExternalInput").ap()
    wq8_d = nc.dram_tensor("wq8", [128, 4, 2, 128], fp8, kind="ExternalInput").ap()
    wk8_d = nc.dram_tensor("wk8", [128, 4, 2, 128], fp8, kind="ExternalInput").ap()
    wv_d = nc.dram_tensor("wv", [D, 128], bf16, kind="ExternalInput").ap()
    wo_d = nc.dram_tensor("wo", [128, D], bf16, kind="ExternalInput").ap()
    # consts: cols 0:128 = upper-tri mask (1 where col >= row), cols 128:192 =
    # 64x64 identity replicated in both partition halves.
    consts_d = nc.dram_tensor("consts", [128, 192], bf16, kind="ExternalInput").ap()
    out_d = nc.dram_tensor("out_pT", [D, BS], bf16, kind="ExternalOutput").ap()

    DC = D // 128   # 8 d-chunks
    OP = DC // 2    # 4 d-chunk pairs for DoubleRow
    NT = S // 128   # 16 key blocks per sequence
    NG = BS // 128  # 32 V groups

    with tile.TileContext(nc) as tc:
        with tc.tile_pool(name="persist", bufs=1) as pp:
            xT8 = pp.tile([128, OP, 2, BS], fp8, tag="xT8")
            qT = pp.tile([128, BS], bf16, tag="qT")
            kT = pp.tile([128, BS], bf16, tag="kT")
            # V in [t, k] layout, padded to 128 columns: col 0 = 1.0 (the
            # ones column makes the attention matmul emit softmax
            # denominators in PSUM partition 0), cols 1:64 = 0, cols
            # 64:128 = V block for s-block g and local head j.
            V_sb = pp.tile([128, NG, HL, 128], bf16, tag="V")
            OT = pp.tile([128, BS], bf16, tag="OT")
            wq8 = pp.tile([128, OP, 2, 128], fp8, tag="wq8")
            wk8 = pp.tile([128, OP, 2, 128], fp8, tag="wk8")
            wv = pp.tile([128, DC, 128], bf16, tag="wv")
            wo = pp.tile([128, D], bf16, tag="wo")
            consts = pp.tile([128, 192], bf16, tag="consts")
            trimask = consts[:, 0:128]
            ident = consts[:, 128:192]  # noqa: F841 (kept for layout parity)
            ones64 = pp.tile([1, 64], f32, tag="ones64")

            # DMA in. The phase-1 critical set (wk8, first xT8 columns, wq8)
            # issues from the ACT queue -- descriptor writes on the sync
            # engine are ~0.6us each and serialize startup otherwise. ACT's
            # exp warmup is emitted after these so it doesn't block them
            # (it waits on consts, which arrives via sync).
            nc.scalar.dma_start(wk8[:], wk8_d[:])
            for op in range(OP):
                nc.scalar.dma_start(xT8[:, op, :, 0:512], xT8_d[:, op, :, 0:512])
            nc.scalar.dma_start(wq8[:], wq8_d[:])
            for op in range(2):
                nc.scalar.dma_start(xT8[:, op, :, 512:2048], xT8_d[:, op, :, 512:2048])
            # sync queue: consts + V-projection inputs early (first V filler
            # runs right after attention starts), bulk batch-1 x later.
            nc.sync.dma_start(consts[:], consts_d[:])
            nc.sync.dma_start(wv[:], wv_d.rearrange("(o p) c -> p o c", p=128))
            for op in range(2, OP):
                nc.sync.dma_start(xT8[:, op, :, 512:2048], xT8_d[:, op, :, 512:2048])
            nc.vector.memset(ones64[:], 1.0)
            # Preheat the ACT exp table so the first real exp doesn't pay
            # the table-load latency mid-pipeline.
            warmup = pp.tile([1, 8], bf16, tag="warmup")
            nc.scalar.activation(warmup[:], consts[0:1, 0:8], Exp, scale=SCALE)

            # ---- Phase 1: Q^T / K^T fp8 DoubleRow projections, batch 0 ----
            # Batch 1's projection chunks are deferred into the attention
            # loop as filler (they're only needed from the third head on).
            # unit order: kT s0 first (first scores block needs it), then
            # qT s0..s3 (rhs of the first scores), then the rest of kT; each
            # unit copies to SBUF immediately so attention starts gap-free.
            with tc.tile_pool(name="ph1psum", bufs=8, space="PSUM") as ph1:
                units = [(wk8, kT, 0), (wq8, qT, 0), (wq8, qT, 1),
                         (wq8, qT, 2), (wq8, qT, 3), (wk8, kT, 1),
                         (wk8, kT, 2), (wk8, kT, 3)]
                for ui, (w_sb, dst, s) in enumerate(units):
                    ps1 = ph1.tile([128, 512], f32, tag="ph1", name="ph1")
                    for op in range(OP):
                        nc.tensor.matmul(
                            ps1[:],
                            lhsT=w_sb[:, op, :, :],
                            rhs=xT8[:, op, :, 512 * s : 512 * (s + 1)],
                            start=(op == 0),
                            stop=(op == OP - 1),
                            perf_mode=DR,
                        )
                    if ui % 2 == 0:
                        nc.scalar.copy(dst[:, 512 * s : 512 * (s + 1)], ps1[:])
                    else:
                        nc.vector.tensor_copy(dst[:, 512 * s : 512 * (s + 1)], ps1[:])

            # V_sb constants; emitted after phase-1 so these DVE ops
            # don't delay the urgent qT/kT copies.
            nc.vector.memset(V_sb[:, :, :, 0:HD], 0.0)
            nc.vector.memset(V_sb[:, :, :, 0:1], 1.0)

            # ---- Phases 2+3: V projection (streamed xTg), attention, and
            # output projection, all interleaved on the PE. ----
            with (
                tc.tile_pool(name="po", bufs=2, space="PSUM") as po_pool,
                tc.tile_pool(name="ps", bufs=2, space="PSUM") as ps_pool,
                tc.tile_pool(name="aux", bufs=2, space="PSUM") as aux_pool,
                tc.tile_pool(name="expp", bufs=26) as exp_pool,
                tc.tile_pool(name="xtg", bufs=8) as xtg_pool,
                tc.tile_pool(name="recip", bufs=3) as rc_pool,
                tc.tile_pool(name="pbcast", bufs=3) as pb_pool,
                tc.tile_pool(name="onum", bufs=3) as on_pool,
                tc.tile_pool(name="ph4out", bufs=2) as ph4o,
            ):
                xtg_tiles = {}

                def fetch_group(g):
                    if g >= NG:
                        return
                    t = xtg_pool.tile([128, DC, 128], bf16, tag="xtg", name="xtg")
                    nc.sync.dma_start(t[:], xTg_d[:, g, :, :])
                    xtg_tiles[g] = t

                def emit_v(vps, g, tag="pv"):
                    # V directly in [t, k] layout: lhsT = xTg s-block
                    # (stationary), rhs = wv: psum[s, c] = V block for both
                    # local heads side by side.
                    fetch_group(g + 6)
                    src = xtg_tiles.pop(g)
                    pv = vps.tile([128, 128], f32, tag=tag, name="pv")
                    for o in range(DC):
                        nc.tensor.matmul(
                            pv[:],
                            lhsT=src[:, o, :],
                            rhs=wv[:, o, :],
                            start=(o == 0),
                            stop=(o == DC - 1),
                        )
                    nc.vector.tensor_copy(
                        V_sb[:, g, :, HD : 2 * HD],
                        pv[:].rearrange("p (j k) -> p j k", j=HL),
                    )

                # prefetch the first V groups before attention begins;
                # batch-1 x and wo are issued mid-bh0 (below) so they don't
                # starve the xTg group stream on the sync queue
                for g in range(6):
                    fetch_group(g)

                # ---- causal attention per (b, local head) ----
                # Quarter-major deferred AV (see v1): scores/exp stream in kj
                # order, every exp tile kept in SBUF, each 512-wide o^T
                # quarter accumulated as one dense burst then normalized.
                def emit_block(b, j, kj, p0, p1):
                    qTh = qT[64 * j : 64 * (j + 1), S * b : S * (b + 1)]
                    kTh = kT[64 * j : 64 * (j + 1), S * b : S * (b + 1)]
                    s_lo = 128 * kj
                    w = p1 - p0
                    ps = ps_pool.tile([128, 1024], f32, tag="ps", name="ps")
                    for c0 in range(0, w, 512):
                        c1 = min(c0 + 512, w)
                        nc.tensor.matmul(
                            ps[:, c0:c1],
                            lhsT=kTh[:, s_lo : s_lo + 128],
                            rhs=qTh[:, p0 + c0 : p0 + c1],
                            start=True,
                            stop=True,
                        )
                    et = exp_pool.tile([128, 1024], bf16, tag="expT", name="et")
                    nc.scalar.activation(et[:, 0:w], ps[:, 0:w], Exp, scale=SCALE)
                    if p0 == s_lo:
                        # diagonal 128x128: keep only s' >= t
                        nc.vector.tensor_mul(et[:, 0:128], et[:, 0:128], trimask[:])
                    return et

                def emit_quarter(b, j, q, ets):
                    kj_last = 4 * q + 3
                    pq = po_pool.tile([128, 512], f32, tag="po", name="pq")
                    for k2 in range(kj_last + 1):
                        a0 = max(512 * q, 128 * k2)
                        a1 = 512 * (q + 1)
                        for p0, p1, et in ets[k2]:
                            if p0 <= a0 < p1:
                                break
                        else:
                            raise AssertionError("no piece")
                        nc.tensor.matmul(
                            pq[:, a0 - 512 * q : a1 - 512 * q],
                            lhsT=V_sb[:, NT * b + k2, j, :],
                            rhs=et[:, a0 - p0 : a1 - p0],
                            start=(k2 == 0),
                            stop=(k2 == kj_last),
                        )
                    return pq

                def emit_normalize(b, j, q, pq):
                    # o^T[k, s] / denom[s] for quarter q. Copy the numerator
                    # to SBUF (frees the po slot), fast-reciprocal the
                    # denominator row (PSUM partition 0), broadcast it
                    # across 64 partitions via a K=1 matmul, then a single
                    # one-PSUM-operand multiply into OT.
                    onum = on_pool.tile([64, 512], f32, tag="onum", name="onum")
                    nc.scalar.copy(onum[:], pq[HD : 2 * HD, :])
                    rc = rc_pool.tile([1, 512], f32, tag="rc", name="rc")
                    nc.vector.reciprocal_approx_fast(rc[:], pq[0:1, :])
                    # broadcast on gpsimd (idle engine) instead of a K=1 PE
                    # matmul: removes 512 PE columns per quarter and takes
                    # pb out of PSUM
                    pb = pb_pool.tile([64, 512], f32, tag="pb", name="pb")
                    nc.gpsimd.partition_broadcast(pb[:], rc[:])
                    nc.vector.tensor_mul(
                        OT[64 * j : 64 * (j + 1),
                           S * b + 512 * q : S * b + 512 * (q + 1)],
                        onum[:],
                        pb[:],
                    )

                out_r = out_d.rearrange("(o p) s -> p o s", p=128)

                def emit_ph4_nb(b, nb, rush=False):
                    # partial projection for 512 columns of batch b:
                    # out_pT[:, cols] = wo.T @ OT[:, cols]. rush=True (the
                    # kernel's last chunk): two half-stages with the casts
                    # alternating ACT/DVE so the tail isn't serialized on
                    # one engine.
                    for half in range(2 if rush else 1):
                        dcs = range(half * 4, half * 4 + 4) if rush else range(DC)
                        stage = ph4o.tile([128, DC, 512], bf16, tag="o4", name="o4")
                        for i, dc in enumerate(dcs):
                            pp4 = aux_pool.tile([128, 512], f32, tag="aux", name="pp4")
                            nc.tensor.matmul(
                                pp4[:],
                                lhsT=wo[:, 128 * dc : 128 * (dc + 1)],
                                rhs=OT[:, S * b + 512 * nb : S * b + 512 * (nb + 1)],
                                start=True,
                                stop=True,
                            )
                            if rush and i % 2 == 0:
                                nc.scalar.copy(stage[:, dc, :], pp4[:])
                            else:
                                nc.vector.tensor_copy(stage[:, dc, :], pp4[:])
                        rows = (slice(half * 4, half * 4 + 4) if rush
                                else slice(0, DC))
                        nc.sync.dma_start(
                            out_r[:, rows, S * b + 512 * nb : S * b + 512 * (nb + 1)],
                            stage[:, rows, :],
                        )

                def emit_qk1(w_sb, dst, sc):
                    # one batch-1 projection chunk: 8 fp8 DoubleRow matmuls
                    pk = aux_pool.tile([128, 512], f32, tag="aux", name="pk")
                    for op in range(OP):
                        nc.tensor.matmul(
                            pk[:],
                            lhsT=w_sb[:, op, :, :],
                            rhs=xT8[:, op, :, 512 * sc : 512 * (sc + 1)],
                            start=(op == 0),
                            stop=(op == OP - 1),
                            perf_mode=DR,
                        )
                    nc.vector.tensor_copy(dst[:, 512 * sc : 512 * (sc + 1)], pk[:])

                filler_q = [("v", g) for g in range(0, BS // 256)]
                for sc in range(4, 8):
                    filler_q.append(("qk", wq8, qT, sc))
                    filler_q.append(("qk", wk8, kT, sc))
                filler_q += [("v", g) for g in range(BS // 256, BS // 128)]

                def emit_filler():
                    if not filler_q:
                        return
                    u = filler_q.pop(0)
                    if u[0] == "qk":
                        emit_qk1(u[1], u[2], u[3])
                    else:
                        emit_v(aux_pool, u[1], tag="aux")

                ph4_queue = []      # (b, nb) chunks awaiting emission
                ph4_state = None    # (b, nb, stage, next_dc)

                def emit_ph4_step():
                    # one dc-chunk of a pending output-projection unit
                    nonlocal ph4_state
                    if ph4_state is None:
                        if not ph4_queue:
                            return
                        b4, nb4 = ph4_queue.pop(0)
                        stage = ph4o.tile([128, DC, 512], bf16, tag="o4", name="o4")
                        ph4_state = (b4, nb4, stage, 0)
                    b4, nb4, stage, dc = ph4_state
                    pp4 = aux_pool.tile([128, 512], f32, tag="aux", name="pp4")
                    nc.tensor.matmul(
                        pp4[:],
                        lhsT=wo[:, 128 * dc : 128 * (dc + 1)],
                        rhs=OT[:, S * b4 + 512 * nb4 : S * b4 + 512 * (nb4 + 1)],
                        start=True,
                        stop=True,
                    )
                    nc.vector.tensor_copy(stage[:, dc, :], pp4[:])
                    if dc == DC - 1:
                        nc.sync.dma_start(
                            out_r[:, :, S * b4 + 512 * nb4 : S * b4 + 512 * (nb4 + 1)],
                            stage[:],
                        )
                        ph4_state = None
                    else:
                        ph4_state = (b4, nb4, stage, dc + 1)

                for bh in range(B * HL):
                    b, j = bh // HL, bh % HL
                    ets = {}
                    for kj in range(NT):
                        s_lo = 128 * kj
                        pieces = ([(s_lo, 1024), (1024, 2048)] if s_lo < 1024
                                  else [(s_lo, 2048)])
                        ets[kj] = [(p0, p1, emit_block(b, j, kj, p0, p1))
                                   for p0, p1 in pieces]
                        # quarter q's AV burst is deferred by one block so
                        # its final exp tile is ready when the burst reaches
                        # it (quarter 3 naturally gets this slack from the
                        # next head's first blocks)
                        if kj % 4 == 0 and kj > 0:
                            q = kj // 4 - 1
                            pq = emit_quarter(b, j, q, ets)
                            emit_normalize(b, j, q, pq)
                            if bh == 3:
                                ph4_queue.append((1, q))
                        if bh == 0 and kj == 6:
                            # batch-1 x for the qk1 fillers (needed from
                            # bh1); bh0's group prefetches got the head
                            # start on the sync queue
                            for op in range(OP):
                                nc.sync.dma_start(
                                    xT8[:, op, :, 2048:4096],
                                    xT8_d[:, op, :, 2048:4096])
                        if bh == 0 and kj == 10:
                            nc.sync.dma_start(wo[:], wo_d[:])
                        if bh == 2 and kj % 4 == 0 and kj > 0:
                            # batch-0 output projection as bh2 filler (OT
                            # batch-0 columns completed during bh1),
                            # spread one dc-chunk per kj step below
                            ph4_queue.append((0, kj // 4 - 1))
                        if bh >= 2:
                            emit_ph4_step()
                            emit_ph4_step()
                        if bh <= 1:
                            # one filler unit per kj in bh0 (paced to the V
                            # deadlines), two per kj in bh1 to drain
                            emit_filler()
                            if bh == 1:
                                emit_filler()
                    q = 3
                    pq = emit_quarter(b, j, q, ets)
                    emit_normalize(b, j, q, pq)
                    if bh == 1:
                        # anything batch-1 still pending must land before bh2
                        while filler_q:
                            emit_filler()
                    if bh == 2:
                        ph4_queue.append((0, 3))
                    if bh == 3:
                        while ph4_queue or ph4_state is not None:
                            emit_ph4_step()
                        emit_ph4_nb(1, 3, rush=True)

    nc.compile()
    return nc


def get_nc():
    if "nc" not in _CACHE:
        _CACHE["nc"] = _build_kernel()
    return _CACHE["nc"]


def make_in_maps(x, Wq, Wk, Wv, Wo):
    """Host-side sharding: per-core input dict (numpy)."""
    x = np.asarray(x, np.float32)
    Wq = np.asarray(Wq, np.float32)
    Wk = np.asarray(Wk, np.float32)
    Wv = np.asarray(Wv, np.float32)
    Wo = np.asarray(Wo, np.float32)
    xT = np.ascontiguousarray(x.transpose(2, 0, 1).reshape(D, BS))
    # x quantized to bf16 first (the v1 baseline numerics), then to e4m3 for
    # the fp8 Q/K path so device and host casts agree.
    xTb = xT.astype(BF16)
    # xT8[p, op, i, s] = xT[128*(2*op+i)+p, s]
    xT8 = np.ascontiguousarray(
        xTb.astype(np.float32).reshape(4, 2, 128, BS).transpose(2, 0, 1, 3)
    ).astype(E4M3)
    # xTg[p, g, o, c] = xT[128*o+p, 128*g+c]
    xTg = np.ascontiguousarray(
        xTb.reshape(D // 128, 128, BS // 128, 128).transpose(1, 2, 0, 3)
    )

    in_maps = []
    for c in range(NCORES):
        h0 = HL * c

        def pack8(W):
            # [HL, D, HD] -> [128, 4, 2, 128] e4m3 pair layout, both heads
            # packed in the last dim (j*64+m)
            Wl = W[h0 : h0 + HL].astype(BF16).astype(np.float32)
            return np.ascontiguousarray(
                Wl.reshape(HL, 4, 2, 128, HD).transpose(3, 1, 2, 0, 4)
                .reshape(128, 4, 2, 128)
            ).astype(E4M3)

        wv_l = np.ascontiguousarray(
            Wv[h0 : h0 + HL].transpose(1, 0, 2).reshape(D, HL * HD)
        ).astype(BF16)
        in_maps.append(
            {
                "xT8": xT8,
                "xTg": xTg,
                "wq8": pack8(Wq),
                "wk8": pack8(Wk),
                "wv": wv_l,
                "wo": np.ascontiguousarray(Wo[128 * c : 128 * (c + 1), :]).astype(BF16),
                "consts": _make_consts(),
            }
        )
    return in_maps


def _make_consts():
    if "consts" not in _CACHE:
        tri = (np.arange(128)[None, :] >= np.arange(128)[:, None]).astype(np.float32)
        eye = np.eye(64, dtype=np.float32)
        c = np.zeros((128, 192), np.float32)
        c[:, 0:128] = tri
        c[0:64, 128:192] = eye
        c[64:128, 128:192] = eye
        _CACHE["consts"] = c.astype(BF16)
    return _CACHE["consts"]


def combine_partials(partials, bo):
    acc = np.zeros((D, BS), np.float32)
    for p in partials:
        acc += np.asarray(p, np.float32)
    out = acc.reshape(D, B, S).transpose(1, 2, 0) + np.asarray(bo, np.float32)[None, None, :]
    return np.ascontiguousarray(out.astype(np.float32))


def kernel(x, Wq, Wk, Wv, Wo, bo):
    from concourse.bass_utils import run_bass_kernel_spmd

    nc = get_nc()
    in_maps = make_in_maps(x, Wq, Wk, Wv, Wo)
    res = run_bass_kernel_spmd(nc, in_maps, core_ids=list(range(NCORES)))
    partials = [r["out_pT"] for r in res.results]
    return combine_partials(partials, bo)
